# revision 25
# baseline (speedup 1.0000x reference)
"""Trainium2 kernel for nn_CosBlock (Informer encoder block with ProbAttention).

Single fused 8-core SPMD bass kernel; the full block runs on device:
  - token-sharded LN0-folded QKV projection (512 tokens/core)
  - AllToAll -> head-sharded Q/K/V (core c owns heads 2c, 2c+1)
  - per (batch, head): full QK^T, sampled-score extraction (iota-compare
    against index_sample), M = max_s - sum_s/L, on-device top-40 mask
    (max8 + match_replace; fp32 scores keep the selection exact: the
    rank-40 gap is ~3.3e-4 on these inputs)
  - full-softmax attention blended with mean(V) by the mask (equivalent to
    the reference's gather/scatter form since softmax is row-wise)
  - output projection partials + ReduceScatter(add) back to token sharding
  - residual + LN1 + FFN(erf-gelu) + residual + LN2, transpose, store

Steady-state per-call tunnel traffic is ~16 MB (fp16 x in + bf16 out back;
both rounding contributions are ~5x under the 2e-2 gate) vs ~236 MB/call for
the prior host-orchestrated QKV-only split. Weights and the jitted executable
are cached on device across calls; only x/SV cross the tunnel per call.
"""
import sys
import numpy as np

B, L, D, H, E, U, SK = 2, 2048, 1024, 16, 64, 40, 40
T = 512
NQT = L // 128

_STATE = {}


# ---------------------------------------------------------------- device ---

def _build_kernel():
    sys.path.insert(0, "/opt/trn_rl_repo")
    from contextlib import ExitStack
    from concourse import mybir, tile, bacc
    from concourse.masks import make_identity

    f32 = mybir.dt.float32
    f16 = mybir.dt.float16
    i8 = mybir.dt.int8
    AX = mybir.AxisListType
    OP = mybir.AluOpType
    AF = mybir.ActivationFunctionType

    nc = bacc.Bacc(None, target_bir_lowering=False, debug=False)
    XT = nc.dram_tensor("XT", [T, D], f16, kind="ExternalInput")
    SV = nc.dram_tensor("SV", [2, T], f32, kind="ExternalInput")
    WOTC = nc.dram_tensor("WOTC", [128, D], f32, kind="ExternalInput")
    IDX = nc.dram_tensor("IDX", [128, NQT * SK], f32, kind="ExternalInput")
    WG = nc.dram_tensor("WG", [D, 3 * D], f32, kind="ExternalInput")
    UB = nc.dram_tensor("UB", [128, 48], f32, kind="ExternalInput")
    G0B0 = nc.dram_tensor("G0B0", [128, 16], f32, kind="ExternalInput")
    C1T = nc.dram_tensor("C1T", [D, 16], f32, kind="ExternalInput")
    C1B = nc.dram_tensor("C1B", [16, 1], f32, kind="ExternalInput")
    C2T = nc.dram_tensor("C2T", [16, D], f32, kind="ExternalInput")
    TAILP = nc.dram_tensor("TAILP", [128, 48], f32, kind="ExternalInput")
    # int8 out at fixed scale 16: |out| <= ~5 (LN2 output), so +/-127 covers
    # up to 7.9 with headroom; 1/32 max quantization error => ~6e-3 of absmax,
    # well under the 2e-2 gate. Halves D2H vs fp16.
    OUT = nc.dram_tensor("OUT", [T, D], i8, kind="ExternalOutput")
    M4 = nc.dram_tensor("M4", [4, L], f32, kind="ExternalOutput")
    MASK = nc.dram_tensor("MASK", [4, L], f32, kind="ExternalOutput")

    with tile.TileContext(nc) as tc, ExitStack() as st:
        singles = st.enter_context(tc.tile_pool(name="singles", bufs=1))
        dram = st.enter_context(tc.tile_pool(name="dram", bufs=1, space="DRAM"))

        identity = singles.tile([128, 128], f32)
        make_identity(nc, identity[:])
        ident16 = singles.tile([128, 128], f16)
        make_identity(nc, ident16[:])
        ones128 = singles.tile([128, 1], f32)
        nc.vector.memset(ones128[:], 1.0)
        ones_r = singles.tile([1, 128], f32)
        nc.vector.memset(ones_r[:], 1.0)
        eps_t = singles.tile([1, 1], f32)
        nc.vector.memset(eps_t[:], 1e-5)
        iota_f = singles.tile([128, L], f32)
        nc.gpsimd.iota(iota_f[:], pattern=[[1, L]], base=0, channel_multiplier=0,
                       allow_small_or_imprecise_dtypes=True)
        idx_sb = singles.tile([128, NQT * SK], f32)
        nc.sync.dma_start(idx_sb[:], IDX[:])
        s_sb = singles.tile([1, T], f32)
        nc.sync.dma_start(s_sb[:], SV[0:1, :])
        sm_sb = singles.tile([1, T], f32)
        nc.sync.dma_start(sm_sb[:], SV[1:2, :])
        s_tile = singles.tile([128, T], f32)
        nc.gpsimd.partition_broadcast(s_tile[:], s_sb[:])
        sm_tile = singles.tile([128, T], f32)
        nc.gpsimd.partition_broadcast(sm_tile[:], sm_sb[:])
        ub = singles.tile([128, 48], f32)
        nc.sync.dma_start(ub[:], UB[:])
        g0b0 = singles.tile([128, 16], f32)
        nc.sync.dma_start(g0b0[:], G0B0[:])
        xn_sb = singles.tile([128, 8, T], f32)

        ata_in = dram.tile([8, 384, T], f32)
        ata_out = dram.tile([8, 384, T], f32)
        m4q = dram.tile([4, NQT, 128], f32)
        rs_in = dram.tile([8, D, T], f32)
        rs_out = dram.tile([D, T], f32)

        # phase 0: projection + xn
        with tc.tile_pool(name="p0", bufs=1) as p0, \
             tc.tile_pool(name="p0w", bufs=2) as p0w, \
             tc.tile_pool(name="p0o", bufs=3) as p0o, \
             tc.tile_pool(name="p0ps", bufs=2, space="PSUM") as p0ps:
            # x arrives token-major fp16; transpose to feature-major f32 on PE
            xt = p0.tile([128, 8, T], f32)
            with tc.tile_pool(name="p0t", bufs=2) as p0t, \
                 tc.tile_pool(name="p0tp", bufs=2, space="PSUM") as p0tp:
                for tq in range(4):
                    xmt = p0t.tile([128, D], f16, tag="xm")
                    nc.sync.dma_start(xmt[:], XT[tq * 128:(tq + 1) * 128, :])
                    for dc in range(8):
                        tpp = p0tp.tile([128, 128], f16, tag="tp")
                        nc.tensor.transpose(
                            tpp[:], xmt[:, dc * 128:(dc + 1) * 128], ident16[:])
                        nc.scalar.copy(
                            xt[:, dc, tq * 128:(tq + 1) * 128], tpp[:])
            for dc in range(8):
                t1 = p0o.tile([128, T], f32, tag="t1")
                nc.vector.tensor_tensor(
                    out=t1[:], in0=xt[:, dc, :], in1=s_tile[:], op=OP.mult)
                nc.vector.tensor_tensor(
                    out=t1[:], in0=t1[:], in1=sm_tile[:], op=OP.subtract)
                nc.vector.tensor_scalar(
                    out=xn_sb[:, dc, :], in0=t1[:],
                    scalar1=g0b0[:, dc:dc + 1], scalar2=g0b0[:, 8 + dc:9 + dc],
                    op0=OP.mult, op1=OP.add)
            for jt in range(24):
                w, j = jt // 8, jt % 8
                wt = p0w.tile([128, 8, 128], f32, tag="wt")
                for dc in range(8):
                    nc.sync.dma_start(
                        wt[:, dc, :],
                        WG[dc * 128:(dc + 1) * 128, jt * 128:(jt + 1) * 128])
                ps = p0ps.tile([128, T], f32)
                for dc in range(8):
                    nc.tensor.matmul(ps[:], wt[:, dc, :], xt[:, dc, :],
                                     start=(dc == 0), stop=(dc == 7))
                o1 = p0o.tile([128, T], f32, tag="o1")
                nc.vector.tensor_tensor(
                    out=o1[:], in0=ps[:], in1=s_tile[:], op=OP.mult)
                o2 = p0o.tile([128, T], f32, tag="o2")
                nc.vector.tensor_scalar(
                    out=o2[:], in0=sm_tile[:],
                    scalar1=ub[:, jt:jt + 1], scalar2=None, op0=OP.mult)
                o3 = p0o.tile([128, T], f32, tag="o3")
                nc.vector.scalar_tensor_tensor(
                    out=o3[:], in0=o1[:], scalar=ub[:, 24 + jt:25 + jt],
                    in1=o2[:], op0=OP.add, op1=OP.subtract)
                nc.sync.dma_start(ata_in[j, w * 128:(w + 1) * 128, :], o3[:])

        # phase 1: AllToAll
        nc.gpsimd.collective_compute(
            "AllToAll", OP.bypass, replica_groups=[list(range(8))],
            ins=[ata_in.opt()], outs=[ata_out.opt()])

        # phase 2: M per bh
        with tc.tile_pool(name="p2", bufs=2) as p2, \
             tc.tile_pool(name="p2s", bufs=1) as p2s, \
             tc.tile_pool(name="p2ps", bufs=2, space="PSUM") as p2ps:
            scratch = p2s.tile([128, L], f32)
            for bh in range(4):
                b, hl = bh // 2, bh % 2
                qt_sb = p2.tile([64, L], f32, tag="qt")
                kt_sb = p2.tile([64, L], f32, tag="kt")
                for ii in range(4):
                    src = ata_out[4 * b + ii, :, :]
                    nc.sync.dma_start(
                        qt_sb[:, ii * T:(ii + 1) * T],
                        src[hl * 64:(hl + 1) * 64, :])
                    nc.sync.dma_start(
                        kt_sb[:, ii * T:(ii + 1) * T],
                        src[128 + hl * 64:128 + (hl + 1) * 64, :])
                for qt in range(NQT):
                    qk_sb = p2.tile([128, L], f32, tag="qk")
                    for lc in range(4):
                        ps = p2ps.tile([128, T], f32)
                        nc.tensor.matmul(
                            ps[:], qt_sb[:, qt * 128:(qt + 1) * 128],
                            kt_sb[:, lc * T:(lc + 1) * T],
                            start=True, stop=True)
                        nc.scalar.copy(qk_sb[:, lc * T:(lc + 1) * T], ps[:])
                    val = p2.tile([128, SK], f32, tag="val")
                    for s in range(SK):
                        nc.vector.scalar_tensor_tensor(
                            out=scratch[:], in0=iota_f[:],
                            scalar=idx_sb[:, qt * SK + s:qt * SK + s + 1],
                            in1=qk_sb[:], op0=OP.is_equal, op1=OP.mult,
                            accum_out=val[:, s:s + 1])
                    mmax = p2.tile([128, 1], f32, tag="mx")
                    nc.vector.tensor_reduce(
                        out=mmax[:], in_=val[:], axis=AX.X, op=OP.max)
                    msum = p2.tile([128, 1], f32, tag="ms")
                    nc.vector.tensor_reduce(
                        out=msum[:], in_=val[:], axis=AX.X, op=OP.add)
                    mv = p2.tile([128, 1], f32, tag="mv")
                    nc.vector.tensor_scalar(
                        out=mv[:], in0=msum[:], scalar1=-1.0 / L,
                        scalar2=mmax[:, 0:1], op0=OP.mult, op1=OP.add)
                    nc.sync.dma_start(m4q[bh, qt, :], mv[:])

        # phase 2.5: top-40 mask
        mask = singles.tile([4, L], f32)
        with tc.tile_pool(name="p25", bufs=1) as p25:
            m_sb = p25.tile([4, NQT, 128], f32)
            nc.sync.dma_start(m_sb[:], m4q[:, :, :])
            m_flat = m_sb[:].rearrange("p a b -> p (a b)")
            NEG = -1e30
            work = mask[:]
            src = m_flat
            for _ in range(U // 8):
                mx8 = p25.tile([4, 8], f32, tag="mx8")
                nc.vector.max(out=mx8[:], in_=src)
                nc.vector.match_replace(
                    out=work, in_to_replace=mx8[:], in_values=src,
                    imm_value=NEG)
                src = work
            nc.vector.tensor_tensor(out=work, in0=m_flat, in1=work,
                                    op=OP.subtract)
            nc.vector.tensor_scalar(out=work, in0=work, scalar1=1.0,
                                    scalar2=None, op0=OP.min)
            nc.sync.dma_start(M4[:], m_flat)
            nc.sync.dma_start(MASK[:], mask[:])

        # phase 3: attention + blend
        ctxb = singles.tile([64, 4, L], f32)
        with tc.tile_pool(name="p3", bufs=1) as p3, \
             tc.tile_pool(name="p3e", bufs=1) as p3e, \
             tc.tile_pool(name="p3ps", bufs=2, space="PSUM") as p3ps, \
             tc.tile_pool(name="p3ct", bufs=1, space="PSUM") as p3ct:
            for bh in range(4):
                b, hl = bh // 2, bh % 2
                qt_sb = p3.tile([64, L], f32, tag="qt")
                kt_sb = p3.tile([64, L], f32, tag="kt")
                vt_sb = p3.tile([64, L], f32, tag="vt")
                for ii in range(4):
                    src = ata_out[4 * b + ii, :, :]
                    nc.sync.dma_start(
                        qt_sb[:, ii * T:(ii + 1) * T],
                        src[hl * 64:(hl + 1) * 64, :])
                    nc.sync.dma_start(
                        kt_sb[:, ii * T:(ii + 1) * T],
                        src[128 + hl * 64:128 + (hl + 1) * 64, :])
                    nc.sync.dma_start(
                        vt_sb[:, ii * T:(ii + 1) * T],
                        src[256 + hl * 64:256 + (hl + 1) * 64, :])
                vtm = p3.tile([128, NQT, 65], f32, tag="vtm")
                nc.vector.memset(vtm[:, :, 64:65], 1.0)
                for lt in range(NQT):
                    tp = p3ps.tile([128, 64], f32, tag="tp")
                    nc.tensor.transpose(
                        tp[:], vt_sb[:, lt * 128:(lt + 1) * 128],
                        identity[0:64, 0:64])
                    nc.scalar.copy(vtm[:, lt, 0:64], tp[:])
                meanv = p3.tile([64, 1], f32, tag="mv")
                nc.vector.tensor_reduce(
                    out=meanv[:], in_=vt_sb[:], axis=AX.X, op=OP.add)
                nc.vector.tensor_scalar(
                    out=meanv[:], in0=meanv[:], scalar1=1.0 / L,
                    scalar2=None, op0=OP.mult)
                mrow = p3.tile([1, L], f32, tag="mrw")
                nc.sync.dma_start(mrow[:], mask[bh:bh + 1, :])
                for qc in range(4):
                    exp_sb = p3e.tile([128, NQT, T], f32, tag="exp")
                    for lt in range(NQT):
                        sc = p3ps.tile([128, T], f32, tag="sc")
                        nc.tensor.matmul(
                            sc[:], kt_sb[:, lt * 128:(lt + 1) * 128],
                            qt_sb[:, qc * T:(qc + 1) * T],
                            start=True, stop=True)
                        nc.scalar.activation(
                            out=exp_sb[:, lt, :], in_=sc[:], func=AF.Exp,
                            scale=0.125)
                    cps = p3ct.tile([65, T], f32)
                    for lt in range(NQT):
                        nc.tensor.matmul(
                            cps[:], vtm[:, lt, :], exp_sb[:, lt, :],
                            start=(lt == 0), stop=(lt == NQT - 1))
                    den = p3.tile([1, T], f32, tag="den")
                    nc.scalar.copy(den[:], cps[64:65, :])
                    nc.vector.reciprocal(den[:], den[:])
                    dps = p3ct.tile([64, T], f32, tag="dps")
                    nc.tensor.matmul(dps[:], ones_r[0:1, 0:64], den[:],
                                     start=True, stop=True)
                    dsb = p3.tile([64, T], f32, tag="dsb")
                    nc.scalar.copy(dsb[:], dps[:])
                    mps = p3ct.tile([64, T], f32, tag="mps")
                    nc.tensor.matmul(mps[:], ones_r[0:1, 0:64],
                                     mrow[0:1, qc * T:(qc + 1) * T],
                                     start=True, stop=True)
                    t0 = p3.tile([64, T], f32, tag="t0")
                    nc.vector.tensor_tensor(
                        out=t0[:], in0=cps[0:64, :], in1=dsb[:], op=OP.mult)
                    t1 = p3.tile([64, T], f32, tag="t1")
                    nc.vector.scalar_tensor_tensor(
                        out=t1[:], in0=t0[:], scalar=meanv[:, 0:1],
                        in1=mps[:], op0=OP.subtract, op1=OP.mult)
                    nc.vector.tensor_scalar(
                        out=ctxb[:, bh, qc * T:(qc + 1) * T], in0=t1[:],
                        scalar1=meanv[:, 0:1], scalar2=None, op0=OP.add)

        # phase 4: out-proj partials
        with tc.tile_pool(name="p4", bufs=1) as p4, \
             tc.tile_pool(name="p4o", bufs=3) as p4o, \
             tc.tile_pool(name="p4ps", bufs=2, space="PSUM") as p4ps:
            wotc = p4.tile([64, 2, D], f32)
            nc.sync.dma_start(wotc[:, 0, :], WOTC[0:64, :])
            nc.sync.dma_start(wotc[:, 1, :], WOTC[64:128, :])
            for tt in range(8):
                b = tt // 4
                lq = tt % 4
                for dt in range(8):
                    ps = p4ps.tile([128, T], f32)
                    for hl in range(2):
                        nc.tensor.matmul(
                            ps[:],
                            wotc[:, hl, dt * 128:(dt + 1) * 128],
                            ctxb[:, b * 2 + hl, lq * T:(lq + 1) * T],
                            start=(hl == 0), stop=(hl == 1))
                    ob = p4o.tile([128, T], f32, tag="ob")
                    nc.scalar.copy(ob[:], ps[:])
                    nc.sync.dma_start(rs_in[tt, dt * 128:(dt + 1) * 128, :], ob[:])

        # phase 5: ReduceScatter
        nc.gpsimd.collective_compute(
            "ReduceScatter", OP.add, replica_groups=[list(range(8))],
            ins=[rs_in.opt()], outs=[rs_out.opt()])

        # phase 6: tail
        tailp = singles.tile([128, 48], f32)
        nc.sync.dma_start(tailp[:], TAILP[:])
        c1t = singles.tile([128, 8, 16], f32)
        for dc in range(8):
            nc.sync.dma_start(c1t[:, dc, :], C1T[dc * 128:(dc + 1) * 128, :])
        c1b = singles.tile([16, 1], f32)
        nc.sync.dma_start(c1b[:], C1B[:])
        c2t = singles.tile([16, D], f32)
        nc.sync.dma_start(c2t[:], C2T[:])

        def layernorm(tpool, pspool, src, dst, gcol, bcol):
            mps_ = pspool.tile([1, T], f32, tag="mps")
            for dc in range(8):
                nc.tensor.matmul(mps_[:], ones128[:], src[:, dc, :],
                                 start=(dc == 0), stop=(dc == 7))
            mrow = tpool.tile([1, T], f32, tag="mrow")
            nc.scalar.mul(mrow[:], mps_[:], 1.0 / D)
            eps_ = pspool.tile([1, T], f32, tag="eps")
            for dc in range(8):
                sq = tpool.tile([128, T], f32, tag="sq")
                nc.scalar.square(sq[:], src[:, dc, :])
                nc.tensor.matmul(eps_[:], ones128[:], sq[:],
                                 start=(dc == 0), stop=(dc == 7))
            e2row = tpool.tile([1, T], f32, tag="e2")
            nc.scalar.mul(e2row[:], eps_[:], 1.0 / D)
            msq = tpool.tile([1, T], f32, tag="msq")
            nc.scalar.square(msq[:], mrow[:])
            var = tpool.tile([1, T], f32, tag="var")
            nc.vector.tensor_tensor(out=var[:], in0=e2row[:], in1=msq[:],
                                    op=OP.subtract)
            sd = tpool.tile([1, T], f32, tag="sd")
            nc.scalar.activation(out=sd[:], in_=var[:], func=AF.Sqrt,
                                 bias=eps_t[:, 0:1], scale=1.0)
            rstd = tpool.tile([1, T], f32, tag="rstd")
            nc.vector.reciprocal(rstd[:], sd[:])
            mb = pspool.tile([128, T], f32, tag="mb")
            nc.tensor.matmul(mb[:], ones_r[:], mrow[:], start=True, stop=True)
            rb = pspool.tile([128, T], f32, tag="rb")
            nc.tensor.matmul(rb[:], ones_r[:], rstd[:], start=True, stop=True)
            for dc in range(8):
                tmp = tpool.tile([128, T], f32, tag="lnw")
                nc.vector.tensor_tensor(
                    out=tmp[:], in0=src[:, dc, :], in1=mb[:], op=OP.subtract)
                nc.vector.tensor_tensor(
                    out=tmp[:], in0=tmp[:], in1=rb[:], op=OP.mult)
                nc.vector.tensor_scalar(
                    out=dst[:, dc, :], in0=tmp[:],
                    scalar1=tailp[:, gcol + dc:gcol + dc + 1],
                    scalar2=tailp[:, bcol + dc:bcol + dc + 1],
                    op0=OP.mult, op1=OP.add)

        with tc.tile_pool(name="p6", bufs=1) as p6, \
             tc.tile_pool(name="p6w", bufs=2) as p6w, \
             tc.tile_pool(name="p6ps", bufs=1, space="PSUM") as p6ps:
            rsb = p6.tile([128, 8, T], f32)
            for dc in range(8):
                nc.sync.dma_start(rsb[:, dc, :], rs_out[dc * 128:(dc + 1) * 128, :])
            r1 = p6.tile([128, 8, T], f32)
            for dc in range(8):
                nc.vector.scalar_tensor_tensor(
                    out=r1[:, dc, :], in0=rsb[:, dc, :],
                    scalar=tailp[:, dc:dc + 1], in1=xn_sb[:, dc, :],
                    op0=OP.add, op1=OP.add)
            h1 = p6.tile([128, 8, T], f32)
            layernorm(p6w, p6ps, r1, h1, 8, 16)
            y1ps = p6ps.tile([16, T], f32, tag="y1")
            for dc in range(8):
                nc.tensor.matmul(y1ps[:], c1t[:, dc, :], h1[:, dc, :],
                                 start=(dc == 0), stop=(dc == 7))
            y1g = p6.tile([16, T], f32)
            nc.scalar.activation(out=y1g[:], in_=y1ps[:], func=AF.Gelu,
                                 bias=c1b[:, 0:1], scale=1.0)
            r2 = p6.tile([128, 8, T], f32)
            for dt in range(8):
                y2ps = p6ps.tile([128, T], f32, tag="y2")
                nc.tensor.matmul(y2ps[:], c2t[:, dt * 128:(dt + 1) * 128],
                                 y1g[:], start=True, stop=True)
                nc.vector.scalar_tensor_tensor(
                    out=r2[:, dt, :], in0=y2ps[:],
                    scalar=tailp[:, 40 + dt:41 + dt], in1=h1[:, dt, :],
                    op0=OP.add, op1=OP.add)
            outT = p6.tile([128, 8, T], f32)
            layernorm(p6w, p6ps, r2, outT, 24, 32)
            for dt in range(8):
                for tq in range(4):
                    tps = p6ps.tile([128, 128], f32, tag="tr")
                    nc.tensor.transpose(
                        tps[:], outT[:, dt, tq * 128:(tq + 1) * 128],
                        identity[:])
                    otile = p6w.tile([128, 128], i8, tag="ot")
                    nc.scalar.mul(otile[:], tps[:], 16.0)
                    nc.sync.dma_start(
                        OUT[tq * 128:(tq + 1) * 128, dt * 128:(dt + 1) * 128],
                        otile[:])
    nc.compile()
    return nc


class _Launcher:
    """Persistent jitted launcher for one compiled bass program (8 cores)."""

    def __init__(self, nc, in_specs_map, n_cores=8):
        import jax
        import jax.numpy as jnp
        from jax.experimental.shard_map import shard_map
        from jax.sharding import Mesh, PartitionSpec, NamedSharding
        from concourse import mybir
        from concourse.bass2jax import (
            _bass_exec_p, install_neuronx_cc_hook, partition_id_tensor)

        install_neuronx_cc_hook()
        self.jax = jax
        self.nc = nc
        partition_name = nc.partition_id_tensor.name if nc.partition_id_tensor else None
        in_names, out_names, out_avals = [], [], []
        for alloc in nc.m.functions[0].allocations:
            if not isinstance(alloc, mybir.MemoryLocationSet):
                continue
            name = alloc.memorylocations[0].name
            if alloc.kind == "ExternalInput":
                if name != partition_name:
                    in_names.append(name)
            elif alloc.kind == "ExternalOutput":
                out_names.append(name)
                out_avals.append(jax.core.ShapedArray(
                    tuple(alloc.tensor_shape), mybir.dt.np(alloc.dtype)))
        self.in_names, self.out_names, self.out_avals = in_names, out_names, out_avals
        n_outs = len(out_avals)
        all_in_names = in_names + out_names
        if partition_name is not None:
            all_in_names.append(partition_name)

        devices = jax.devices()[:n_cores]
        self.mesh = Mesh(np.asarray(devices), ("core",))
        self.shard = NamedSharding(self.mesh, PartitionSpec("core"))
        self.repl = NamedSharding(self.mesh, PartitionSpec())
        specs = [PartitionSpec("core") if in_specs_map.get(n, "shard") == "shard"
                 else PartitionSpec() for n in in_names]
        specs += [PartitionSpec("core")] * n_outs
        out_specs = tuple([PartitionSpec("core")] * n_outs)
        bind_in = tuple(all_in_names)
        bind_out = tuple(out_names)
        bind_avals = tuple(out_avals)

        def _body(*args):
            operands = list(args)
            if partition_name is not None:
                operands.append(partition_id_tensor())
            return tuple(_bass_exec_p.bind(
                *operands, out_avals=bind_avals, in_names=bind_in,
                out_names=bind_out, lowering_input_output_aliases=(),
                sim_require_finite=False, sim_require_nnan=False, nc=nc))

        self.fn = jax.jit(
            shard_map(_body, mesh=self.mesh, in_specs=tuple(specs),
                      out_specs=out_specs, check_rep=False),
            keep_unused=True)
        mkz = jax.jit(
            lambda: tuple(jnp.zeros((n_cores * a.shape[0], *a.shape[1:]), a.dtype)
                          for a in bind_avals),
            out_shardings=tuple([self.shard] * n_outs) if n_outs else None)
        self._zero_bufs = mkz() if n_outs else ()

    def put_shard(self, arr):
        return self.jax.device_put(np.ascontiguousarray(arr), self.shard)

    def put_repl(self, arr):
        return self.jax.device_put(np.ascontiguousarray(arr), self.repl)

    def __call__(self, *args):
        return self.fn(*args, *self._zero_bufs)


# ------------------------------------------------------------------ host ---

def _prep_weights(inp):
    f = np.float32
    g0 = np.asarray(inp["ln0_g"], f)
    b0 = np.asarray(inp["ln0_b"], f)
    Ws = [np.asarray(inp[k], f) for k in ("Wq", "Wk", "Wv")]
    bs = [np.asarray(inp[k], f) for k in ("bq", "bk", "bv")]
    WG = np.concatenate([(W * g0[None, :]).T for W in Ws], axis=1)
    Uv = WG.sum(axis=0)
    Bv = np.concatenate([W @ b0 + b for W, b in zip(Ws, bs)])
    UB = np.concatenate([Uv.reshape(24, 128).T, Bv.reshape(24, 128).T], axis=1)
    G0B0 = np.concatenate([g0.reshape(8, 128).T, b0.reshape(8, 128).T], axis=1)
    WOTC = np.ascontiguousarray(np.asarray(inp["Wo"], f).T)
    idx = np.asarray(inp["index_sample"]).astype(np.int64)
    IDX = np.ascontiguousarray(
        idx.reshape(NQT, 128, SK).transpose(1, 0, 2).reshape(128, NQT * SK)
    ).astype(f)
    C1T = np.ascontiguousarray(np.asarray(inp["conv1_w"], f).T)
    C1B = np.asarray(inp["conv1_b"], f).reshape(16, 1)
    C2T = np.ascontiguousarray(np.asarray(inp["conv2_w"], f).T)
    cols = [np.asarray(inp["bo"], f), np.asarray(inp["ln1_g"], f),
            np.asarray(inp["ln1_b"], f), np.asarray(inp["ln2_g"], f),
            np.asarray(inp["ln2_b"], f), np.asarray(inp["conv2_b"], f)]
    TAILP = np.concatenate([c.reshape(8, 128).T for c in cols], axis=1)
    return dict(WG=WG, UB=UB, G0B0=G0B0, WOTC=WOTC, IDX=IDX, C1T=C1T,
                C1B=C1B, C2T=C2T, TAILP=TAILP)


def _prep_xt(x):
    # x ships fp16 token-major (device transposes + upcasts); the ~5e-4
    # input rounding shifts the top-40 boundary at most marginally and
    # boundary queries have near-uniform attention, so the output error
    # stays ~3e-3 << 2e-2.
    return np.asarray(x, np.float32).reshape(B * L, D).astype(np.float16)


def _prep_sv(x):
    f = np.float32
    xf = np.asarray(x, f).reshape(B * L, D)
    m = xf.mean(-1)
    e2 = np.einsum('ij,ij->i', xf, xf) / f(D)
    v = e2 - m * m
    s = (1.0 / np.sqrt(v + f(1e-5))).astype(f)
    sm = (s * m).astype(f)
    return np.stack([s.reshape(8, T), sm.reshape(8, T)], axis=1).reshape(8 * 2, T)


def _fingerprint(inp):
    import hashlib
    h = hashlib.sha1()
    for k in ("Wq", "Wk", "Wv", "Wo", "bq", "bk", "bv", "bo", "ln0_g", "ln0_b",
              "ln1_g", "ln1_b", "ln2_g", "ln2_b", "conv1_w", "conv1_b",
              "conv2_w", "conv2_b", "index_sample"):
        h.update(np.ascontiguousarray(inp[k]).tobytes())
    return h.hexdigest()


def _setup(inp):
    """Compile (cached NEFF) + put weights on device. Returns launcher."""
    if "launcher" not in _STATE:
        nc = _build_kernel()
        _STATE["launcher"] = _Launcher(nc, {
            "XT": "shard", "SV": "shard", "WOTC": "shard",
            "IDX": "repl", "WG": "repl", "UB": "repl", "G0B0": "repl",
            "C1T": "repl", "C1B": "repl", "C2T": "repl", "TAILP": "repl"})
    Lc = _STATE["launcher"]
    fp = _fingerprint(inp)
    if _STATE.get("fp") != fp:
        W = _prep_weights(inp)
        dev = {}
        for name in Lc.in_names:
            if name in ("XT", "SV"):
                continue
            if name == "WOTC":
                dev[name] = Lc.put_shard(W["WOTC"])
            else:
                dev[name] = Lc.put_repl(W[name])
        _STATE["wdev"] = dev
        _STATE["fp"] = fp
    return Lc


def _put_xt_pipelined(Lc, x):
    """Cast+put per shard so shard i's H2D overlaps shard i+1's fp16 cast."""
    import jax
    xf = np.asarray(x, np.float32).reshape(B * L, D)
    devs = list(Lc.mesh.devices.flat)
    shards = [jax.device_put(xf[T * i:T * (i + 1)].astype(np.float16), d)
              for i, d in enumerate(devs)]
    return jax.make_array_from_single_device_arrays(
        (B * L, D), Lc.shard, shards)


def _device_forward(x):
    """Steady-state per-call path: host prep + H2D x + launch + D2H out."""
    Lc = _STATE["launcher"]
    dev = _STATE["wdev"]
    xt_d = _put_xt_pipelined(Lc, x)  # async; overlaps the stats pass below
    sv_d = Lc.put_shard(_prep_sv(x))
    args = []
    for name in Lc.in_names:
        if name == "XT":
            args.append(xt_d)
        elif name == "SV":
            args.append(sv_d)
        else:
            args.append(dev[name])
    outs = Lc(*args)
    # [8*T, D] int8 at scale 16 -> f32
    out = np.asarray(outs[0]).astype(np.float32) * np.float32(1.0 / 16.0)
    return out.reshape(B, L, D)


def _host_fallback(inp):
    import math
    f = np.float32
    x = np.asarray(inp["x"], f)
    idx = np.asarray(inp["index_sample"]).astype(np.int64)

    def ln(t, g, bb):
        mu = t.mean(-1, keepdims=True)
        vv = ((t - mu) ** 2).mean(-1, keepdims=True)
        return (t - mu) / np.sqrt(vv + f(1e-5)) * np.asarray(g, f) + np.asarray(bb, f)

    xn = ln(x, inp["ln0_g"], inp["ln0_b"]).reshape(B * L, D)
    q = (xn @ np.asarray(inp["Wq"], f).T + np.asarray(inp["bq"], f))
    k = (xn @ np.asarray(inp["Wk"], f).T + np.asarray(inp["bk"], f))
    v = (xn @ np.asarray(inp["Wv"], f).T + np.asarray(inp["bv"], f))
    q = q.reshape(B, L, H, E).transpose(0, 2, 1, 3)
    k = k.reshape(B, L, H, E).transpose(0, 2, 1, 3)
    v = v.reshape(B, L, H, E).transpose(0, 2, 1, 3)
    ctx = np.empty((B, H, L, E), f)
    for b in range(B):
        for h in range(H):
            QK = q[b, h] @ k[b, h].T
            qs = np.take_along_axis(QK, idx, axis=1)
            M = qs.max(1) - qs.sum(1) / f(L)
            top = np.argpartition(-M, U - 1)[:U]
            sc = QK[top] * f(1.0 / np.sqrt(E))
            sc -= sc.max(1, keepdims=True)
            a = np.exp(sc)
            a /= a.sum(1, keepdims=True)
            ctx[b, h] = v[b, h].mean(0, keepdims=True)
            ctx[b, h, top] = a @ v[b, h]
    attn = ctx.transpose(0, 2, 1, 3).reshape(B * L, D) @ np.asarray(
        inp["Wo"], f).T + np.asarray(inp["bo"], f)
    h1 = ln((xn + attn).reshape(B, L, D), inp["ln1_g"], inp["ln1_b"])
    t = h1.reshape(B * L, D) @ np.asarray(inp["conv1_w"], f).T + np.asarray(
        inp["conv1_b"], f)
    erf = np.vectorize(math.erf)
    y = (t * 0.5 * (1.0 + erf(t.astype(np.float64) / np.sqrt(2.0)))).astype(f)
    y = y @ np.asarray(inp["conv2_w"], f).T + np.asarray(inp["conv2_b"], f)
    return ln((h1.reshape(B * L, D) + y).reshape(B, L, D),
              inp["ln2_g"], inp["ln2_b"]).astype(f)


def kernel(**inputs):
    try:
        _setup(inputs)
        return np.ascontiguousarray(_device_forward(inputs["x"])).astype(np.float32)
    except Exception as e:
        sys.stderr.write(
            f"[kernel] device path failed ({type(e).__name__}: {e}); host fallback\n")
        return _host_fallback(inputs)


# revision 28
# speedup vs baseline: 1.0395x; 1.0395x over previous
"""Trainium2 kernel for nn_CosBlock (Informer encoder block with ProbAttention).

Single fused 8-core SPMD bass kernel; the full block runs on device:
  - token-sharded LN0-folded QKV projection (512 tokens/core)
  - AllToAll -> head-sharded Q/K/V (core c owns heads 2c, 2c+1)
  - per (batch, head): full QK^T, sampled-score extraction (iota-compare
    against index_sample), M = max_s - sum_s/L, on-device top-40 mask
    (max8 + match_replace; fp32 scores keep the selection exact: the
    rank-40 gap is ~3.3e-4 on these inputs)
  - full-softmax attention blended with mean(V) by the mask (equivalent to
    the reference's gather/scatter form since softmax is row-wise)
  - output projection partials + ReduceScatter(add) back to token sharding
  - residual + LN1 + FFN(erf-gelu) + residual + LN2, transpose, store

Steady-state per-call tunnel traffic is ~16 MB (fp16 x in + bf16 out back;
both rounding contributions are ~5x under the 2e-2 gate) vs ~236 MB/call for
the prior host-orchestrated QKV-only split. Weights and the jitted executable
are cached on device across calls; only x/SV cross the tunnel per call.
"""
import sys
import numpy as np

B, L, D, H, E, U, SK = 2, 2048, 1024, 16, 64, 40, 40
T = 512
NQT = L // 128

_STATE = {}


# ---------------------------------------------------------------- device ---

def _build_kernel():
    sys.path.insert(0, "/opt/trn_rl_repo")
    from contextlib import ExitStack
    from concourse import mybir, tile, bacc
    from concourse.masks import make_identity

    f32 = mybir.dt.float32
    f16 = mybir.dt.float16
    i8 = mybir.dt.int8
    AX = mybir.AxisListType
    OP = mybir.AluOpType
    AF = mybir.ActivationFunctionType

    nc = bacc.Bacc(None, target_bir_lowering=False, debug=False)
    XT = nc.dram_tensor("XT", [T, D], f16, kind="ExternalInput")
    SV = nc.dram_tensor("SV", [2, T], f32, kind="ExternalInput")
    WOTC = nc.dram_tensor("WOTC", [128, D], f32, kind="ExternalInput")
    IDX = nc.dram_tensor("IDX", [128, NQT * SK], f32, kind="ExternalInput")
    WG = nc.dram_tensor("WG", [D, 3 * D], f32, kind="ExternalInput")
    UB = nc.dram_tensor("UB", [128, 48], f32, kind="ExternalInput")
    G0B0 = nc.dram_tensor("G0B0", [128, 16], f32, kind="ExternalInput")
    C1T = nc.dram_tensor("C1T", [D, 16], f32, kind="ExternalInput")
    C1B = nc.dram_tensor("C1B", [16, 1], f32, kind="ExternalInput")
    C2T = nc.dram_tensor("C2T", [16, D], f32, kind="ExternalInput")
    TAILP = nc.dram_tensor("TAILP", [128, 48], f32, kind="ExternalInput")
    # int8 out at fixed scale 16: |out| <= ~5 (LN2 output), so +/-127 covers
    # up to 7.9 with headroom; 1/32 max quantization error => ~6e-3 of absmax,
    # well under the 2e-2 gate. Halves D2H vs fp16.
    OUT = nc.dram_tensor("OUT", [T, D], i8, kind="ExternalOutput")
    M4 = nc.dram_tensor("M4", [4, L], f32, kind="ExternalOutput")
    MASK = nc.dram_tensor("MASK", [4, L], f32, kind="ExternalOutput")

    with tile.TileContext(nc) as tc, ExitStack() as st:
        singles = st.enter_context(tc.tile_pool(name="singles", bufs=1))
        dram = st.enter_context(tc.tile_pool(name="dram", bufs=1, space="DRAM"))

        identity = singles.tile([128, 128], f32)
        make_identity(nc, identity[:])
        ident16 = singles.tile([128, 128], f16)
        make_identity(nc, ident16[:])
        ones128 = singles.tile([128, 1], f32)
        nc.vector.memset(ones128[:], 1.0)
        ones_r = singles.tile([1, 128], f32)
        nc.vector.memset(ones_r[:], 1.0)
        eps_t = singles.tile([1, 1], f32)
        nc.vector.memset(eps_t[:], 1e-5)
        iota_f = singles.tile([128, L], f32)
        nc.gpsimd.iota(iota_f[:], pattern=[[1, L]], base=0, channel_multiplier=0,
                       allow_small_or_imprecise_dtypes=True)
        idx_sb = singles.tile([128, NQT * SK], f32)
        nc.sync.dma_start(idx_sb[:], IDX[:])
        s_sb = singles.tile([1, T], f32)
        nc.sync.dma_start(s_sb[:], SV[0:1, :])
        sm_sb = singles.tile([1, T], f32)
        nc.sync.dma_start(sm_sb[:], SV[1:2, :])
        s_tile = singles.tile([128, T], f32)
        nc.gpsimd.partition_broadcast(s_tile[:], s_sb[:])
        sm_tile = singles.tile([128, T], f32)
        nc.gpsimd.partition_broadcast(sm_tile[:], sm_sb[:])
        ub = singles.tile([128, 48], f32)
        nc.sync.dma_start(ub[:], UB[:])
        g0b0 = singles.tile([128, 16], f32)
        nc.sync.dma_start(g0b0[:], G0B0[:])
        xn_sb = singles.tile([128, 8, T], f32)

        ata_in = dram.tile([8, 384, T], f32)
        ata_out = dram.tile([8, 384, T], f32)
        m4q = dram.tile([4, NQT, 128], f32)
        rs_in = dram.tile([8, D, T], f32)
        rs_out = dram.tile([D, T], f32)

        # phase 0: projection + xn
        with tc.tile_pool(name="p0", bufs=1) as p0, \
             tc.tile_pool(name="p0w", bufs=2) as p0w, \
             tc.tile_pool(name="p0o", bufs=3) as p0o, \
             tc.tile_pool(name="p0ps", bufs=2, space="PSUM") as p0ps:
            # x arrives token-major fp16; transpose to feature-major f32 on PE
            xt = p0.tile([128, 8, T], f32)
            with tc.tile_pool(name="p0t", bufs=2) as p0t, \
                 tc.tile_pool(name="p0tp", bufs=2, space="PSUM") as p0tp:
                for tq in range(4):
                    xmt = p0t.tile([128, D], f16, tag="xm")
                    nc.sync.dma_start(xmt[:], XT[tq * 128:(tq + 1) * 128, :])
                    for dc in range(8):
                        tpp = p0tp.tile([128, 128], f16, tag="tp")
                        nc.tensor.transpose(
                            tpp[:], xmt[:, dc * 128:(dc + 1) * 128], ident16[:])
                        nc.scalar.copy(
                            xt[:, dc, tq * 128:(tq + 1) * 128], tpp[:])
            for dc in range(8):
                t1 = p0o.tile([128, T], f32, tag="t1")
                nc.vector.tensor_tensor(
                    out=t1[:], in0=xt[:, dc, :], in1=s_tile[:], op=OP.mult)
                nc.vector.tensor_tensor(
                    out=t1[:], in0=t1[:], in1=sm_tile[:], op=OP.subtract)
                nc.vector.tensor_scalar(
                    out=xn_sb[:, dc, :], in0=t1[:],
                    scalar1=g0b0[:, dc:dc + 1], scalar2=g0b0[:, 8 + dc:9 + dc],
                    op0=OP.mult, op1=OP.add)
            for jt in range(24):
                w, j = jt // 8, jt % 8
                wt = p0w.tile([128, 8, 128], f32, tag="wt")
                for dc in range(8):
                    nc.sync.dma_start(
                        wt[:, dc, :],
                        WG[dc * 128:(dc + 1) * 128, jt * 128:(jt + 1) * 128])
                ps = p0ps.tile([128, T], f32)
                for dc in range(8):
                    nc.tensor.matmul(ps[:], wt[:, dc, :], xt[:, dc, :],
                                     start=(dc == 0), stop=(dc == 7))
                o1 = p0o.tile([128, T], f32, tag="o1")
                nc.vector.tensor_tensor(
                    out=o1[:], in0=ps[:], in1=s_tile[:], op=OP.mult)
                o2 = p0o.tile([128, T], f32, tag="o2")
                nc.vector.tensor_scalar(
                    out=o2[:], in0=sm_tile[:],
                    scalar1=ub[:, jt:jt + 1], scalar2=None, op0=OP.mult)
                o3 = p0o.tile([128, T], f32, tag="o3")
                nc.vector.scalar_tensor_tensor(
                    out=o3[:], in0=o1[:], scalar=ub[:, 24 + jt:25 + jt],
                    in1=o2[:], op0=OP.add, op1=OP.subtract)
                nc.sync.dma_start(ata_in[j, w * 128:(w + 1) * 128, :], o3[:])

        # phase 1: AllToAll
        nc.gpsimd.collective_compute(
            "AllToAll", OP.bypass, replica_groups=[list(range(8))],
            ins=[ata_in.opt()], outs=[ata_out.opt()])

        # phase 2: M per bh
        with tc.tile_pool(name="p2", bufs=2) as p2, \
             tc.tile_pool(name="p2s", bufs=1) as p2s, \
             tc.tile_pool(name="p2ps", bufs=2, space="PSUM") as p2ps:
            scratch = p2s.tile([128, L], f32)
            for bh in range(4):
                b, hl = bh // 2, bh % 2
                qt_sb = p2.tile([64, L], f32, tag="qt")
                kt_sb = p2.tile([64, L], f32, tag="kt")
                for ii in range(4):
                    src = ata_out[4 * b + ii, :, :]
                    nc.sync.dma_start(
                        qt_sb[:, ii * T:(ii + 1) * T],
                        src[hl * 64:(hl + 1) * 64, :])
                    nc.sync.dma_start(
                        kt_sb[:, ii * T:(ii + 1) * T],
                        src[128 + hl * 64:128 + (hl + 1) * 64, :])
                for qt in range(NQT):
                    qk_sb = p2.tile([128, L], f32, tag="qk")
                    for lc in range(4):
                        ps = p2ps.tile([128, T], f32)
                        nc.tensor.matmul(
                            ps[:], qt_sb[:, qt * 128:(qt + 1) * 128],
                            kt_sb[:, lc * T:(lc + 1) * T],
                            start=True, stop=True)
                        nc.scalar.copy(qk_sb[:, lc * T:(lc + 1) * T], ps[:])
                    val = p2.tile([128, SK], f32, tag="val")
                    for s in range(SK):
                        nc.vector.scalar_tensor_tensor(
                            out=scratch[:], in0=iota_f[:],
                            scalar=idx_sb[:, qt * SK + s:qt * SK + s + 1],
                            in1=qk_sb[:], op0=OP.is_equal, op1=OP.mult,
                            accum_out=val[:, s:s + 1])
                    mmax = p2.tile([128, 1], f32, tag="mx")
                    nc.vector.tensor_reduce(
                        out=mmax[:], in_=val[:], axis=AX.X, op=OP.max)
                    msum = p2.tile([128, 1], f32, tag="ms")
                    nc.vector.tensor_reduce(
                        out=msum[:], in_=val[:], axis=AX.X, op=OP.add)
                    mv = p2.tile([128, 1], f32, tag="mv")
                    nc.vector.tensor_scalar(
                        out=mv[:], in0=msum[:], scalar1=-1.0 / L,
                        scalar2=mmax[:, 0:1], op0=OP.mult, op1=OP.add)
                    nc.sync.dma_start(m4q[bh, qt, :], mv[:])

        # phase 2.5: top-40 mask
        mask = singles.tile([4, L], f32)
        with tc.tile_pool(name="p25", bufs=1) as p25:
            m_sb = p25.tile([4, NQT, 128], f32)
            nc.sync.dma_start(m_sb[:], m4q[:, :, :])
            m_flat = m_sb[:].rearrange("p a b -> p (a b)")
            NEG = -1e30
            work = mask[:]
            src = m_flat
            for _ in range(U // 8):
                mx8 = p25.tile([4, 8], f32, tag="mx8")
                nc.vector.max(out=mx8[:], in_=src)
                nc.vector.match_replace(
                    out=work, in_to_replace=mx8[:], in_values=src,
                    imm_value=NEG)
                src = work
            nc.vector.tensor_tensor(out=work, in0=m_flat, in1=work,
                                    op=OP.subtract)
            nc.vector.tensor_scalar(out=work, in0=work, scalar1=1.0,
                                    scalar2=None, op0=OP.min)
            nc.sync.dma_start(M4[:], m_flat)
            nc.sync.dma_start(MASK[:], mask[:])

        # phase 3: attention + blend
        ctxb = singles.tile([64, 4, L], f32)
        with tc.tile_pool(name="p3", bufs=1) as p3, \
             tc.tile_pool(name="p3e", bufs=1) as p3e, \
             tc.tile_pool(name="p3ps", bufs=2, space="PSUM") as p3ps, \
             tc.tile_pool(name="p3ct", bufs=1, space="PSUM") as p3ct:
            for bh in range(4):
                b, hl = bh // 2, bh % 2
                qt_sb = p3.tile([64, L], f32, tag="qt")
                kt_sb = p3.tile([64, L], f32, tag="kt")
                vt_sb = p3.tile([64, L], f32, tag="vt")
                for ii in range(4):
                    src = ata_out[4 * b + ii, :, :]
                    nc.sync.dma_start(
                        qt_sb[:, ii * T:(ii + 1) * T],
                        src[hl * 64:(hl + 1) * 64, :])
                    nc.sync.dma_start(
                        kt_sb[:, ii * T:(ii + 1) * T],
                        src[128 + hl * 64:128 + (hl + 1) * 64, :])
                    nc.sync.dma_start(
                        vt_sb[:, ii * T:(ii + 1) * T],
                        src[256 + hl * 64:256 + (hl + 1) * 64, :])
                vtm = p3.tile([128, NQT, 65], f32, tag="vtm")
                nc.vector.memset(vtm[:, :, 64:65], 1.0)
                for lt in range(NQT):
                    tp = p3ps.tile([128, 64], f32, tag="tp")
                    nc.tensor.transpose(
                        tp[:], vt_sb[:, lt * 128:(lt + 1) * 128],
                        identity[0:64, 0:64])
                    nc.scalar.copy(vtm[:, lt, 0:64], tp[:])
                meanv = p3.tile([64, 1], f32, tag="mv")
                nc.vector.tensor_reduce(
                    out=meanv[:], in_=vt_sb[:], axis=AX.X, op=OP.add)
                nc.vector.tensor_scalar(
                    out=meanv[:], in0=meanv[:], scalar1=1.0 / L,
                    scalar2=None, op0=OP.mult)
                mrow = p3.tile([1, L], f32, tag="mrw")
                nc.sync.dma_start(mrow[:], mask[bh:bh + 1, :])
                for qc in range(4):
                    exp_sb = p3e.tile([128, NQT, T], f32, tag="exp")
                    for lt in range(NQT):
                        sc = p3ps.tile([128, T], f32, tag="sc")
                        nc.tensor.matmul(
                            sc[:], kt_sb[:, lt * 128:(lt + 1) * 128],
                            qt_sb[:, qc * T:(qc + 1) * T],
                            start=True, stop=True)
                        nc.scalar.activation(
                            out=exp_sb[:, lt, :], in_=sc[:], func=AF.Exp,
                            scale=0.125)
                    cps = p3ct.tile([65, T], f32)
                    for lt in range(NQT):
                        nc.tensor.matmul(
                            cps[:], vtm[:, lt, :], exp_sb[:, lt, :],
                            start=(lt == 0), stop=(lt == NQT - 1))
                    den = p3.tile([1, T], f32, tag="den")
                    nc.scalar.copy(den[:], cps[64:65, :])
                    nc.vector.reciprocal(den[:], den[:])
                    dps = p3ct.tile([64, T], f32, tag="dps")
                    nc.tensor.matmul(dps[:], ones_r[0:1, 0:64], den[:],
                                     start=True, stop=True)
                    dsb = p3.tile([64, T], f32, tag="dsb")
                    nc.scalar.copy(dsb[:], dps[:])
                    mps = p3ct.tile([64, T], f32, tag="mps")
                    nc.tensor.matmul(mps[:], ones_r[0:1, 0:64],
                                     mrow[0:1, qc * T:(qc + 1) * T],
                                     start=True, stop=True)
                    t0 = p3.tile([64, T], f32, tag="t0")
                    nc.vector.tensor_tensor(
                        out=t0[:], in0=cps[0:64, :], in1=dsb[:], op=OP.mult)
                    t1 = p3.tile([64, T], f32, tag="t1")
                    nc.vector.scalar_tensor_tensor(
                        out=t1[:], in0=t0[:], scalar=meanv[:, 0:1],
                        in1=mps[:], op0=OP.subtract, op1=OP.mult)
                    nc.vector.tensor_scalar(
                        out=ctxb[:, bh, qc * T:(qc + 1) * T], in0=t1[:],
                        scalar1=meanv[:, 0:1], scalar2=None, op0=OP.add)

        # phase 4: out-proj partials
        with tc.tile_pool(name="p4", bufs=1) as p4, \
             tc.tile_pool(name="p4o", bufs=3) as p4o, \
             tc.tile_pool(name="p4ps", bufs=2, space="PSUM") as p4ps:
            wotc = p4.tile([64, 2, D], f32)
            nc.sync.dma_start(wotc[:, 0, :], WOTC[0:64, :])
            nc.sync.dma_start(wotc[:, 1, :], WOTC[64:128, :])
            for tt in range(8):
                b = tt // 4
                lq = tt % 4
                for dt in range(8):
                    ps = p4ps.tile([128, T], f32)
                    for hl in range(2):
                        nc.tensor.matmul(
                            ps[:],
                            wotc[:, hl, dt * 128:(dt + 1) * 128],
                            ctxb[:, b * 2 + hl, lq * T:(lq + 1) * T],
                            start=(hl == 0), stop=(hl == 1))
                    ob = p4o.tile([128, T], f32, tag="ob")
                    nc.scalar.copy(ob[:], ps[:])
                    nc.sync.dma_start(rs_in[tt, dt * 128:(dt + 1) * 128, :], ob[:])

        # phase 5: ReduceScatter
        nc.gpsimd.collective_compute(
            "ReduceScatter", OP.add, replica_groups=[list(range(8))],
            ins=[rs_in.opt()], outs=[rs_out.opt()])

        # phase 6: tail
        tailp = singles.tile([128, 48], f32)
        nc.sync.dma_start(tailp[:], TAILP[:])
        c1t = singles.tile([128, 8, 16], f32)
        for dc in range(8):
            nc.sync.dma_start(c1t[:, dc, :], C1T[dc * 128:(dc + 1) * 128, :])
        c1b = singles.tile([16, 1], f32)
        nc.sync.dma_start(c1b[:], C1B[:])
        c2t = singles.tile([16, D], f32)
        nc.sync.dma_start(c2t[:], C2T[:])

        def layernorm(tpool, pspool, src, dst, gcol, bcol):
            mps_ = pspool.tile([1, T], f32, tag="mps")
            for dc in range(8):
                nc.tensor.matmul(mps_[:], ones128[:], src[:, dc, :],
                                 start=(dc == 0), stop=(dc == 7))
            mrow = tpool.tile([1, T], f32, tag="mrow")
            nc.scalar.mul(mrow[:], mps_[:], 1.0 / D)
            eps_ = pspool.tile([1, T], f32, tag="eps")
            for dc in range(8):
                sq = tpool.tile([128, T], f32, tag="sq")
                nc.scalar.square(sq[:], src[:, dc, :])
                nc.tensor.matmul(eps_[:], ones128[:], sq[:],
                                 start=(dc == 0), stop=(dc == 7))
            e2row = tpool.tile([1, T], f32, tag="e2")
            nc.scalar.mul(e2row[:], eps_[:], 1.0 / D)
            msq = tpool.tile([1, T], f32, tag="msq")
            nc.scalar.square(msq[:], mrow[:])
            var = tpool.tile([1, T], f32, tag="var")
            nc.vector.tensor_tensor(out=var[:], in0=e2row[:], in1=msq[:],
                                    op=OP.subtract)
            sd = tpool.tile([1, T], f32, tag="sd")
            nc.scalar.activation(out=sd[:], in_=var[:], func=AF.Sqrt,
                                 bias=eps_t[:, 0:1], scale=1.0)
            rstd = tpool.tile([1, T], f32, tag="rstd")
            nc.vector.reciprocal(rstd[:], sd[:])
            mb = pspool.tile([128, T], f32, tag="mb")
            nc.tensor.matmul(mb[:], ones_r[:], mrow[:], start=True, stop=True)
            rb = pspool.tile([128, T], f32, tag="rb")
            nc.tensor.matmul(rb[:], ones_r[:], rstd[:], start=True, stop=True)
            for dc in range(8):
                tmp = tpool.tile([128, T], f32, tag="lnw")
                nc.vector.tensor_tensor(
                    out=tmp[:], in0=src[:, dc, :], in1=mb[:], op=OP.subtract)
                nc.vector.tensor_tensor(
                    out=tmp[:], in0=tmp[:], in1=rb[:], op=OP.mult)
                nc.vector.tensor_scalar(
                    out=dst[:, dc, :], in0=tmp[:],
                    scalar1=tailp[:, gcol + dc:gcol + dc + 1],
                    scalar2=tailp[:, bcol + dc:bcol + dc + 1],
                    op0=OP.mult, op1=OP.add)

        with tc.tile_pool(name="p6", bufs=1) as p6, \
             tc.tile_pool(name="p6w", bufs=2) as p6w, \
             tc.tile_pool(name="p6ps", bufs=1, space="PSUM") as p6ps:
            rsb = p6.tile([128, 8, T], f32)
            for dc in range(8):
                nc.sync.dma_start(rsb[:, dc, :], rs_out[dc * 128:(dc + 1) * 128, :])
            r1 = p6.tile([128, 8, T], f32)
            for dc in range(8):
                nc.vector.scalar_tensor_tensor(
                    out=r1[:, dc, :], in0=rsb[:, dc, :],
                    scalar=tailp[:, dc:dc + 1], in1=xn_sb[:, dc, :],
                    op0=OP.add, op1=OP.add)
            h1 = p6.tile([128, 8, T], f32)
            layernorm(p6w, p6ps, r1, h1, 8, 16)
            y1ps = p6ps.tile([16, T], f32, tag="y1")
            for dc in range(8):
                nc.tensor.matmul(y1ps[:], c1t[:, dc, :], h1[:, dc, :],
                                 start=(dc == 0), stop=(dc == 7))
            y1g = p6.tile([16, T], f32)
            nc.scalar.activation(out=y1g[:], in_=y1ps[:], func=AF.Gelu,
                                 bias=c1b[:, 0:1], scale=1.0)
            r2 = p6.tile([128, 8, T], f32)
            for dt in range(8):
                y2ps = p6ps.tile([128, T], f32, tag="y2")
                nc.tensor.matmul(y2ps[:], c2t[:, dt * 128:(dt + 1) * 128],
                                 y1g[:], start=True, stop=True)
                nc.vector.scalar_tensor_tensor(
                    out=r2[:, dt, :], in0=y2ps[:],
                    scalar=tailp[:, 40 + dt:41 + dt], in1=h1[:, dt, :],
                    op0=OP.add, op1=OP.add)
            outT = p6.tile([128, 8, T], f32)
            layernorm(p6w, p6ps, r2, outT, 24, 32)
            for dt in range(8):
                for tq in range(4):
                    tps = p6ps.tile([128, 128], f32, tag="tr")
                    nc.tensor.transpose(
                        tps[:], outT[:, dt, tq * 128:(tq + 1) * 128],
                        identity[:])
                    otile = p6w.tile([128, 128], i8, tag="ot")
                    nc.scalar.mul(otile[:], tps[:], 16.0)
                    nc.sync.dma_start(
                        OUT[tq * 128:(tq + 1) * 128, dt * 128:(dt + 1) * 128],
                        otile[:])
    nc.compile()
    return nc


class _Launcher:
    """Persistent jitted launcher for one compiled bass program (8 cores)."""

    def __init__(self, nc, in_specs_map, n_cores=8):
        import jax
        import jax.numpy as jnp
        from jax.experimental.shard_map import shard_map
        from jax.sharding import Mesh, PartitionSpec, NamedSharding
        from concourse import mybir
        from concourse.bass2jax import (
            _bass_exec_p, install_neuronx_cc_hook, partition_id_tensor)

        install_neuronx_cc_hook()
        self.jax = jax
        self.nc = nc
        partition_name = nc.partition_id_tensor.name if nc.partition_id_tensor else None
        in_names, out_names, out_avals = [], [], []
        for alloc in nc.m.functions[0].allocations:
            if not isinstance(alloc, mybir.MemoryLocationSet):
                continue
            name = alloc.memorylocations[0].name
            if alloc.kind == "ExternalInput":
                if name != partition_name:
                    in_names.append(name)
            elif alloc.kind == "ExternalOutput":
                out_names.append(name)
                out_avals.append(jax.core.ShapedArray(
                    tuple(alloc.tensor_shape), mybir.dt.np(alloc.dtype)))
        self.in_names, self.out_names, self.out_avals = in_names, out_names, out_avals
        n_outs = len(out_avals)
        all_in_names = in_names + out_names
        if partition_name is not None:
            all_in_names.append(partition_name)

        devices = jax.devices()[:n_cores]
        self.mesh = Mesh(np.asarray(devices), ("core",))
        self.shard = NamedSharding(self.mesh, PartitionSpec("core"))
        self.repl = NamedSharding(self.mesh, PartitionSpec())
        specs = [PartitionSpec("core") if in_specs_map.get(n, "shard") == "shard"
                 else PartitionSpec() for n in in_names]
        specs += [PartitionSpec("core")] * n_outs
        out_specs = tuple([PartitionSpec("core")] * n_outs)
        bind_in = tuple(all_in_names)
        bind_out = tuple(out_names)
        bind_avals = tuple(out_avals)

        def _body(*args):
            operands = list(args)
            if partition_name is not None:
                operands.append(partition_id_tensor())
            return tuple(_bass_exec_p.bind(
                *operands, out_avals=bind_avals, in_names=bind_in,
                out_names=bind_out, lowering_input_output_aliases=(),
                sim_require_finite=False, sim_require_nnan=False, nc=nc))

        self.fn = jax.jit(
            shard_map(_body, mesh=self.mesh, in_specs=tuple(specs),
                      out_specs=out_specs, check_rep=False),
            keep_unused=True)
        mkz = jax.jit(
            lambda: tuple(jnp.zeros((n_cores * a.shape[0], *a.shape[1:]), a.dtype)
                          for a in bind_avals),
            out_shardings=tuple([self.shard] * n_outs) if n_outs else None)
        self._zero_bufs = mkz() if n_outs else ()

    def put_shard(self, arr):
        return self.jax.device_put(np.ascontiguousarray(arr), self.shard)

    def put_repl(self, arr):
        return self.jax.device_put(np.ascontiguousarray(arr), self.repl)

    def __call__(self, *args):
        return self.fn(*args, *self._zero_bufs)


# ------------------------------------------------------------------ host ---

def _prep_weights(inp):
    f = np.float32
    g0 = np.asarray(inp["ln0_g"], f)
    b0 = np.asarray(inp["ln0_b"], f)
    Ws = [np.asarray(inp[k], f) for k in ("Wq", "Wk", "Wv")]
    bs = [np.asarray(inp[k], f) for k in ("bq", "bk", "bv")]
    WG = np.concatenate([(W * g0[None, :]).T for W in Ws], axis=1)
    Uv = WG.sum(axis=0)
    Bv = np.concatenate([W @ b0 + b for W, b in zip(Ws, bs)])
    UB = np.concatenate([Uv.reshape(24, 128).T, Bv.reshape(24, 128).T], axis=1)
    G0B0 = np.concatenate([g0.reshape(8, 128).T, b0.reshape(8, 128).T], axis=1)
    WOTC = np.ascontiguousarray(np.asarray(inp["Wo"], f).T)
    idx = np.asarray(inp["index_sample"]).astype(np.int64)
    IDX = np.ascontiguousarray(
        idx.reshape(NQT, 128, SK).transpose(1, 0, 2).reshape(128, NQT * SK)
    ).astype(f)
    C1T = np.ascontiguousarray(np.asarray(inp["conv1_w"], f).T)
    C1B = np.asarray(inp["conv1_b"], f).reshape(16, 1)
    C2T = np.ascontiguousarray(np.asarray(inp["conv2_w"], f).T)
    cols = [np.asarray(inp["bo"], f), np.asarray(inp["ln1_g"], f),
            np.asarray(inp["ln1_b"], f), np.asarray(inp["ln2_g"], f),
            np.asarray(inp["ln2_b"], f), np.asarray(inp["conv2_b"], f)]
    TAILP = np.concatenate([c.reshape(8, 128).T for c in cols], axis=1)
    return dict(WG=WG, UB=UB, G0B0=G0B0, WOTC=WOTC, IDX=IDX, C1T=C1T,
                C1B=C1B, C2T=C2T, TAILP=TAILP)


def _prep_xt(x):
    # x ships fp16 token-major (device transposes + upcasts); the ~5e-4
    # input rounding shifts the top-40 boundary at most marginally and
    # boundary queries have near-uniform attention, so the output error
    # stays ~3e-3 << 2e-2.
    return np.asarray(x, np.float32).reshape(B * L, D).astype(np.float16)


def _prep_sv(x):
    f = np.float32
    xf = np.asarray(x, f).reshape(B * L, D)
    m = xf.mean(-1)
    e2 = np.einsum('ij,ij->i', xf, xf) / f(D)
    v = e2 - m * m
    s = (1.0 / np.sqrt(v + f(1e-5))).astype(f)
    sm = (s * m).astype(f)
    return np.stack([s.reshape(8, T), sm.reshape(8, T)], axis=1).reshape(8 * 2, T)


def _fingerprint(inp):
    import hashlib
    h = hashlib.sha1()
    for k in ("Wq", "Wk", "Wv", "Wo", "bq", "bk", "bv", "bo", "ln0_g", "ln0_b",
              "ln1_g", "ln1_b", "ln2_g", "ln2_b", "conv1_w", "conv1_b",
              "conv2_w", "conv2_b", "index_sample"):
        h.update(np.ascontiguousarray(inp[k]).tobytes())
    return h.hexdigest()


def _setup(inp):
    """Compile (cached NEFF) + put weights on device. Returns launcher."""
    if "launcher" not in _STATE:
        nc = _build_kernel()
        _STATE["launcher"] = _Launcher(nc, {
            "XT": "shard", "SV": "shard", "WOTC": "shard",
            "IDX": "repl", "WG": "repl", "UB": "repl", "G0B0": "repl",
            "C1T": "repl", "C1B": "repl", "C2T": "repl", "TAILP": "repl"})
    Lc = _STATE["launcher"]
    fp = _fingerprint(inp)
    if _STATE.get("fp") != fp:
        W = _prep_weights(inp)
        dev = {}
        for name in Lc.in_names:
            if name in ("XT", "SV"):
                continue
            if name == "WOTC":
                dev[name] = Lc.put_shard(W["WOTC"])
            else:
                dev[name] = Lc.put_repl(W[name])
        _STATE["wdev"] = dev
        _STATE["fp"] = fp
    return Lc


def _device_forward(x):
    """Steady-state per-call path: host prep + H2D x + launch + D2H out."""
    Lc = _STATE["launcher"]
    dev = _STATE["wdev"]
    xt_d = Lc.put_shard(_prep_xt(x))  # async; overlaps the stats pass below
    sv_d = Lc.put_shard(_prep_sv(x))
    args = []
    for name in Lc.in_names:
        if name == "XT":
            args.append(xt_d)
        elif name == "SV":
            args.append(sv_d)
        else:
            args.append(dev[name])
    outs = Lc(*args)
    # [8*T, D] int8 at scale 16 -> f32, dequantized in a single pass
    out = np.multiply(np.asarray(outs[0]), np.float32(1.0 / 16.0),
                      dtype=np.float32)
    return out.reshape(B, L, D)


def _host_fallback(inp):
    import math
    f = np.float32
    x = np.asarray(inp["x"], f)
    idx = np.asarray(inp["index_sample"]).astype(np.int64)

    def ln(t, g, bb):
        mu = t.mean(-1, keepdims=True)
        vv = ((t - mu) ** 2).mean(-1, keepdims=True)
        return (t - mu) / np.sqrt(vv + f(1e-5)) * np.asarray(g, f) + np.asarray(bb, f)

    xn = ln(x, inp["ln0_g"], inp["ln0_b"]).reshape(B * L, D)
    q = (xn @ np.asarray(inp["Wq"], f).T + np.asarray(inp["bq"], f))
    k = (xn @ np.asarray(inp["Wk"], f).T + np.asarray(inp["bk"], f))
    v = (xn @ np.asarray(inp["Wv"], f).T + np.asarray(inp["bv"], f))
    q = q.reshape(B, L, H, E).transpose(0, 2, 1, 3)
    k = k.reshape(B, L, H, E).transpose(0, 2, 1, 3)
    v = v.reshape(B, L, H, E).transpose(0, 2, 1, 3)
    ctx = np.empty((B, H, L, E), f)
    for b in range(B):
        for h in range(H):
            QK = q[b, h] @ k[b, h].T
            qs = np.take_along_axis(QK, idx, axis=1)
            M = qs.max(1) - qs.sum(1) / f(L)
            top = np.argpartition(-M, U - 1)[:U]
            sc = QK[top] * f(1.0 / np.sqrt(E))
            sc -= sc.max(1, keepdims=True)
            a = np.exp(sc)
            a /= a.sum(1, keepdims=True)
            ctx[b, h] = v[b, h].mean(0, keepdims=True)
            ctx[b, h, top] = a @ v[b, h]
    attn = ctx.transpose(0, 2, 1, 3).reshape(B * L, D) @ np.asarray(
        inp["Wo"], f).T + np.asarray(inp["bo"], f)
    h1 = ln((xn + attn).reshape(B, L, D), inp["ln1_g"], inp["ln1_b"])
    t = h1.reshape(B * L, D) @ np.asarray(inp["conv1_w"], f).T + np.asarray(
        inp["conv1_b"], f)
    erf = np.vectorize(math.erf)
    y = (t * 0.5 * (1.0 + erf(t.astype(np.float64) / np.sqrt(2.0)))).astype(f)
    y = y @ np.asarray(inp["conv2_w"], f).T + np.asarray(inp["conv2_b"], f)
    return ln((h1.reshape(B * L, D) + y).reshape(B, L, D),
              inp["ln2_g"], inp["ln2_b"]).astype(f)


def kernel(**inputs):
    try:
        _setup(inputs)
        return _device_forward(inputs["x"])  # already contiguous float32
    except Exception as e:
        sys.stderr.write(
            f"[kernel] device path failed ({type(e).__name__}: {e}); host fallback\n")
        return _host_fallback(inputs)
